# revision 16
# baseline (speedup 1.0000x reference)
"""Content-addressed cache-select kernel for Trainium2 (8 NeuronCores, SPMD).

Problem: out = cached_outputs[idx] where idx is the first row of
`fingerprints` (6x4) exactly equal to the first 4 floats of `x`, else 0.

Strategy (row-parallel over 8 cores):
  - Each core receives its 2048-row shard of all 6 cached slabs plus a
    small staged "meta" vector (fingerprints, the replicated probe tiled
    x6, and index weights) packed on the host.
  - On device: one DMA stages meta into SBUF; the vector engine computes
    the first-match index in 6 small int32 ops (bitwise equality ==
    float equality for these inputs; first match wins, no match -> 0);
    the SP engine loads the index into a register and issues
    dynamic-offset DRAM->DRAM DMAs copying the selected 32MB slab shard
    to the output.
  - The copy (DRAM->DRAM, 64KB descriptors) saturates the ~660GB/s
    per-core HBM read+write ceiling. SDMA engine 15 intermittently runs
    ~18% slow (observed in ~half of profiled runs), so the descriptor
    schedule gives it half-load: part A (contiguous, SP queue) spreads
    17 descriptors to each of the 16 engines; part B (4-row-block
    interleaved AP, ACT queue) gives engines 0-14 another 16. Makespan
    is ~33 descriptors/engine whether or not engine 15 is slow.
"""
import numpy as np

import concourse.bass as bass
import concourse.mybir as mybir
from concourse.bass_utils import run_bass_kernel_spmd

N_CASES = 6
ROWS, COLS = 16384, 4096
N_CORES = 8
RS = ROWS // N_CORES  # rows per core

# Copy split between the two HWDGE queues (SP and Activation issue one DMA
# each). The shapes pin the SDMA engine grouping: part A ([16, 17, 16384]
# after lowering) gives every engine 17 64KB descriptors; part B
# ([15, 16, 16384]) gives engines 0-14 another 16. SDMA engine 15
# intermittently runs ~18% slow (observed in half of profiled runs), so it
# gets half-load; makespan is ~33 descriptors/engine either way.
ROWS_A = 1088  # 16 groups x 68 rows -> 17 x 64KB descriptors per engine
ROWS_B = RS - ROWS_A  # 960 = 15 groups x 64 rows -> 16 x 64KB per engine


def build(rows_a=None, rows_b=None, groups_a=16, groups_b=15):
    rows_a = rows_a or ROWS_A
    rows_b = rows_b if rows_b is not None else ROWS_B
    assert rows_a % groups_a == 0 and rows_b % groups_b == 0
    nc = bass.Bass()
    f32 = mybir.dt.float32
    i32 = mybir.dt.int32

    meta = nc.dram_tensor("meta", [1, 64], i32, kind="ExternalInput")
    cached = nc.dram_tensor("cached", [N_CASES, RS, COLS], f32, kind="ExternalInput")
    out = nc.dram_tensor("out", [RS, COLS], f32, kind="ExternalOutput")

    with (
        nc.sbuf_tensor("stage", [1, 128], i32) as stage,
        nc.Block(no_gpsimd_drain=True) as block,
        nc.semaphore("ssem") as ssem,
        nc.semaphore("vsem") as vsem,
        nc.semaphore("bsem") as bsem,
        nc.semaphore("asem") as asem,
    ):

        @block.sync
        def _(sync):
            sync.dma_start(stage[0:1, 0:64], meta[0:1, 0:64]).then_inc(ssem, 16)

        @block.vector
        def _(vector):
            vector.wait_ge(ssem, 16)
            st = stage
            step = [0]

            def chain(inst):
                step[0] += 1
                inst.then_inc(vsem, 1)
                vector.wait_ge(vsem, step[0])

            # eq[64:88] = (fps == probe_tiled) as int32 0/1 (bitwise equality)
            chain(
                vector.tensor_tensor(
                    st[0:1, 64:88],
                    st[0:1, 0:24],
                    st[0:1, 24:48],
                    mybir.AluOpType.is_equal,
                )
            )
            # all4[88:94] = min over each fingerprint's 4 equality bits
            eq_v = st[0:1, 64:88].rearrange("p (a b) -> p a b", a=6)
            chain(
                vector.tensor_reduce(
                    st[0:1, 88:94], eq_v, mybir.AxisListType.X, mybir.AluOpType.min
                )
            )
            # score[94:100] = all4 * [8,7,6,5,4,3] (weights staged at [48:54])
            chain(
                vector.tensor_tensor(
                    st[0:1, 94:100],
                    st[0:1, 88:94],
                    st[0:1, 48:54],
                    mybir.AluOpType.mult,
                )
            )
            # m[100:101] = max(score) = 8 - first_match (0 if no match).
            # The issuing engines turn m into the index with register ALU
            # ops, which are cheaper than two more DVE ops here.
            chain(
                vector.tensor_reduce(
                    st[0:1, 100:101],
                    st[0:1, 94:100],
                    mybir.AxisListType.X,
                    mybir.AluOpType.max,
                )
            )

        def interleaved(ap, groups, f):
            # [r, COLS] region traversed as [groups, m, f*COLS]: 4-row (64KB)
            # blocks are dealt round-robin to `groups` outer slots, and the
            # strided outer dim survives AP optimization, pinning the SDMA
            # engine grouping to engines 0..groups-1. Same pattern on both
            # sides of the DMA keeps the element mapping the identity.
            if len(ap.shape) == 3:  # dynamic [1, r, COLS] slice of cached
                return ap.rearrange("q (m x f) c -> (q x) m (f c)", x=groups, f=f)
            return ap.rearrange("(m x f) c -> x m (f c)", x=groups, f=f)

        def load_idx(eng, name):
            # idx = (8 - m) & 7: first-match index, no-match m=0 -> 8&7 = 0.
            with eng.register(name) as r:
                eng.reg_load(r, stage[0:1, 100:101])
                eng.reg_alu(r, 8, r, mybir.AluOpType.subtract)
                eng.reg_alu(r, r, 7, mybir.AluOpType.bitwise_and)
                return eng.snap(r, min_val=0, max_val=N_CASES - 1)

        @block.sync
        def _(sync):
            sync.wait_ge(vsem, 4)
            idx = load_idx(sync, "idxr")
            rows = slice(0, rows_a)
            sync.dma_start(
                out[rows, :], cached[bass.ds(idx, 1), rows, :]
            ).then_inc(bsem, 16)
            sync.wait_ge(bsem, 16)

        @block.scalar
        def _(scalar):
            if rows_b == 0:
                return
            scalar.wait_ge(vsem, 4)
            idx2 = load_idx(scalar, "idxa")
            rows = slice(rows_a, RS)
            f = 4 if COLS == 4096 else 1
            scalar.dma_start(
                interleaved(out[rows, :], groups_b, f),
                interleaved(cached[bass.ds(idx2, 1), rows, :], groups_b, f),
            ).then_inc(asem, 16)
            scalar.wait_ge(asem, 16)

    return nc


def make_meta(probe, fps):
    buf = np.zeros((1, 64), dtype=np.int32)
    buf[0, 0:24] = fps.reshape(-1).view(np.int32)
    buf[0, 24:48] = np.tile(probe.reshape(-1), 6).view(np.int32)
    buf[0, 48:54] = np.array([8, 7, 6, 5, 4, 3], dtype=np.int32)
    return buf


def run(inputs, trace=False, **spmd_kwargs):
    x = np.asarray(inputs["x"], dtype=np.float32)
    fingerprints = np.asarray(inputs["fingerprints"], dtype=np.float32)
    cached_outputs = np.asarray(inputs["cached_outputs"], dtype=np.float32)

    nc = build()
    meta = make_meta(x.reshape(-1)[:4], fingerprints)
    in_maps = []
    for c in range(N_CORES):
        shard = np.ascontiguousarray(cached_outputs[:, c * RS : (c + 1) * RS, :])
        in_maps.append({"meta": meta, "cached": shard})

    res = run_bass_kernel_spmd(
        nc, in_maps, list(range(N_CORES)), trace=trace, **spmd_kwargs
    )
    out = np.concatenate([res.results[c]["out"] for c in range(N_CORES)], axis=0)
    return out.astype(np.float32), res


def kernel(**inputs) -> np.ndarray:
    out, _ = run(inputs, trace=False)
    return out


# revision 19
# speedup vs baseline: 1.8587x; 1.8587x over previous
"""Content-addressed cache-select kernel for Trainium2 (8 NeuronCores, SPMD).

Problem: out = cached_outputs[idx] where idx is the first row of
`fingerprints` (6x4) exactly equal to the first 4 floats of `x`, else 0.

Strategy (row-parallel over 8 cores):
  - Each core receives its 2048-row shard of all 6 cached slabs plus a
    small staged "meta" vector (fingerprints, the replicated probe tiled
    x6, and index weights) packed on the host.
  - On device: one DMA stages meta into SBUF; the vector engine reduces
    the fingerprint comparison to m = 8 - first_match in 4 small int32
    ops (bitwise equality == float equality for these inputs; first
    match wins, no match -> m=0); the SP and ACT engines each load m,
    finish idx = (8-m)&7 with two register ALU ops, and issue
    dynamic-offset DRAM->DRAM DMAs copying the selected 32MB slab shard
    to the output.
  - The copy (DRAM->DRAM, 64KB descriptors) saturates the ~660GB/s
    per-core HBM read+write ceiling. SDMA engine 15 intermittently runs
    ~18% slow (observed in ~half of profiled runs), so the descriptor
    schedule gives it half-load: part A (contiguous, SP queue) spreads
    17 descriptors to each of the 16 engines; part B (4-row-block
    interleaved AP, ACT queue) gives engines 0-14 another 16. Makespan
    is ~33 descriptors/engine whether or not engine 15 is slow.
"""
import numpy as np

import concourse.bass as bass
import concourse.mybir as mybir
from concourse.bass_utils import run_bass_kernel_spmd

N_CASES = 6
ROWS, COLS = 16384, 4096
N_CORES = 8
RS = ROWS // N_CORES  # rows per core

# Copy split between the two HWDGE queues (SP and Activation issue one DMA
# each). The shapes pin the SDMA engine grouping: part A ([16, 17, 16384]
# after lowering) gives every engine 17 64KB descriptors; part B
# ([15, 16, 16384]) gives engines 0-14 another 16. SDMA engine 15
# intermittently runs ~18% slow (observed in half of profiled runs), so it
# gets half-load; makespan is ~33 descriptors/engine either way.
ROWS_A = 1088  # 16 groups x 68 rows -> 17 x 64KB descriptors per engine
ROWS_B = RS - ROWS_A  # 960 = 15 groups x 64 rows -> 16 x 64KB per engine


def build(rows_a=None, rows_b=None, groups_a=16, groups_b=15):
    rows_a = rows_a or ROWS_A
    rows_b = rows_b if rows_b is not None else ROWS_B
    assert rows_a % groups_a == 0 and rows_b % groups_b == 0
    nc = bass.Bass()
    f32 = mybir.dt.float32
    i32 = mybir.dt.int32

    meta = nc.dram_tensor("meta", [1, 64], i32, kind="ExternalInput")
    cached = nc.dram_tensor("cached", [N_CASES, RS, COLS], f32, kind="ExternalInput")
    out = nc.dram_tensor("out", [RS, COLS], f32, kind="ExternalOutput")

    with (
        nc.sbuf_tensor("stage", [1, 128], i32) as stage,
        nc.Block(no_gpsimd_drain=True) as block,
        nc.semaphore("ssem") as ssem,
        nc.semaphore("vsem") as vsem,
        nc.semaphore("bsem") as bsem,
        nc.semaphore("asem") as asem,
    ):

        @block.sync
        def _(sync):
            sync.dma_start(stage[0:1, 0:64], meta[0:1, 0:64]).then_inc(ssem, 16)

        @block.vector
        def _(vector):
            vector.wait_ge(ssem, 16)
            st = stage
            step = [0]

            def chain(inst):
                step[0] += 1
                inst.then_inc(vsem, 1)
                vector.wait_ge(vsem, step[0])

            # eq[64:88] = (fps == probe_tiled) as int32 0/1 (bitwise equality)
            chain(
                vector.tensor_tensor(
                    st[0:1, 64:88],
                    st[0:1, 0:24],
                    st[0:1, 24:48],
                    mybir.AluOpType.is_equal,
                )
            )
            # all4[88:94] = min over each fingerprint's 4 equality bits
            eq_v = st[0:1, 64:88].rearrange("p (a b) -> p a b", a=6)
            chain(
                vector.tensor_reduce(
                    st[0:1, 88:94], eq_v, mybir.AxisListType.X, mybir.AluOpType.min
                )
            )
            # score[94:100] = all4 * [8,7,6,5,4,3] (weights staged at [48:54])
            chain(
                vector.tensor_tensor(
                    st[0:1, 94:100],
                    st[0:1, 88:94],
                    st[0:1, 48:54],
                    mybir.AluOpType.mult,
                )
            )
            # m[100:101] = max(score) = 8 - first_match (0 if no match).
            # The issuing engines turn m into the index with register ALU
            # ops, which are cheaper than two more DVE ops here.
            chain(
                vector.tensor_reduce(
                    st[0:1, 100:101],
                    st[0:1, 94:100],
                    mybir.AxisListType.X,
                    mybir.AluOpType.max,
                )
            )

        def interleaved(ap, groups, f):
            # [r, COLS] region traversed as [groups, m, f*COLS]: 4-row (64KB)
            # blocks are dealt round-robin to `groups` outer slots, and the
            # strided outer dim survives AP optimization, pinning the SDMA
            # engine grouping to engines 0..groups-1. Same pattern on both
            # sides of the DMA keeps the element mapping the identity.
            if len(ap.shape) == 3:  # dynamic [1, r, COLS] slice of cached
                return ap.rearrange("q (m x f) c -> (q x) m (f c)", x=groups, f=f)
            return ap.rearrange("(m x f) c -> x m (f c)", x=groups, f=f)

        def load_idx(eng, name):
            # idx = (8 - m) & 7: first-match index, no-match m=0 -> 8&7 = 0.
            with eng.register(name) as r:
                eng.reg_load(r, stage[0:1, 100:101])
                eng.reg_alu(r, 8, r, mybir.AluOpType.subtract)
                eng.reg_alu(r, r, 7, mybir.AluOpType.bitwise_and)
                return eng.snap(r, min_val=0, max_val=N_CASES - 1)

        @block.sync
        def _(sync):
            sync.wait_ge(vsem, 4)
            idx = load_idx(sync, "idxr")
            rows = slice(0, rows_a)
            sync.dma_start(
                out[rows, :], cached[bass.ds(idx, 1), rows, :]
            ).then_inc(bsem, 16)
            sync.wait_ge(bsem, 16)

        @block.scalar
        def _(scalar):
            if rows_b == 0:
                return
            scalar.wait_ge(vsem, 4)
            idx2 = load_idx(scalar, "idxa")
            rows = slice(rows_a, RS)
            f = 4 if COLS == 4096 else 1
            scalar.dma_start(
                interleaved(out[rows, :], groups_b, f),
                interleaved(cached[bass.ds(idx2, 1), rows, :], groups_b, f),
            ).then_inc(asem, 16)
            scalar.wait_ge(asem, 16)

    return nc


def make_meta(probe, fps):
    buf = np.zeros((1, 64), dtype=np.int32)
    buf[0, 0:24] = fps.reshape(-1).view(np.int32)
    buf[0, 24:48] = np.tile(probe.reshape(-1), 6).view(np.int32)
    buf[0, 48:54] = np.array([8, 7, 6, 5, 4, 3], dtype=np.int32)
    return buf


def run(inputs, trace=False, **spmd_kwargs):
    x = np.asarray(inputs["x"], dtype=np.float32)
    fingerprints = np.asarray(inputs["fingerprints"], dtype=np.float32)
    cached_outputs = np.asarray(inputs["cached_outputs"], dtype=np.float32)

    nc = build()
    meta = make_meta(x.reshape(-1)[:4], fingerprints)
    in_maps = []
    for c in range(N_CORES):
        shard = np.ascontiguousarray(cached_outputs[:, c * RS : (c + 1) * RS, :])
        in_maps.append({"meta": meta, "cached": shard})

    res = run_bass_kernel_spmd(
        nc, in_maps, list(range(N_CORES)), trace=trace, **spmd_kwargs
    )
    out = np.concatenate([res.results[c]["out"] for c in range(N_CORES)], axis=0)
    return out.astype(np.float32), res


def kernel(**inputs) -> np.ndarray:
    out, _ = run(inputs, trace=False)
    return out
